# revision 8
# baseline (speedup 1.0000x reference)
"""Trainium2 Bass kernel for nn_DiscretePolicy (gnn_message_passing).

Reference computation:
  Xn = batchnorm(X)  (training-mode, biased var, eps=1e-5)
  ent = Xn[:, 4:].reshape(B, 100, 2)
  me = leaky_relu(ent @ W_me.T + b_me); me_out = mean_k(me)      # [B, 64]
  h = leaky_relu([Xn[:, :4], me_out] @ W1.T + b1)
  h = leaky_relu(h @ W2.T + b2)
  out = softmax(h @ W3.T + b3)

Strategy (8-way batch-parallel, 2048 rows/core), v2:
  - X is gathered/cast/transposed on the HOST into a feature-major bf16
    layout XT [128, 2 regions, 2048]: entity pair p sits at partitions
    32*(p%4) + 4*((p//4)%8) .. +3 of region p//32 (quadrant-balanced for
    tile_position concurrency); head features 0..3 at region 1 parts
    124..127.  Batch columns are permuted so the post-softmax transpose
    writes OUT with 512B-contiguous DMA descriptors.
  - BatchNorm stats: per-core sums via DVE accum_out (sum x, sum x^2 per
    partition), then a tiny 8-core AllReduce (2KB) through HBM bounce
    buffers; rsqrt via reciprocal+sqrt.  Normalization applied in-place.
    ACT sqrt/exp table sets are preloaded in the AllReduce shadow.
  - leaky_relu(z) decomposed as alpha*z + (1-alpha)*relu(z).  All alpha
    (linear) paths through the 3 MLP layers are folded analytically into
    direct matmuls on (R_raw, xtA, xtB); only R = sum_k relu(z_k + b_me)
    and the two hidden relu branches are materialized:
      * entity matmuls: groups of 2 pairs into a [128, 1024] fp32 PSUM
        slot (2 banks), 3 slots => 3-deep software pipeline
      * relu+bias split across ScalarE / VectorE (14:11), fp8 y out
      * pooling over entities: one fp8 DoubleRow PE matmul per group,
        accumulated into acc [64, 512] PSUM
  - softmax via PE transpose to batch-major, Exp + reciprocal.
"""

import sys
import numpy as np

sys.path.insert(0, "/opt/trn_rl_repo")

import ml_dtypes

B_FULL, D, H, A = 16384, 204, 64, 32
NCORES = 8
BL = B_FULL // NCORES          # 2048 rows per core
NBT = 4                        # batch tiles per core
NT = BL // NBT                 # 512 columns per batch tile
K_ENT = 100                    # entities
NPAIR = 50                     # entity pairs
GW = 2                         # pairs per PSUM group
NGRP = NPAIR // GW             # 25 groups per batch tile
NSCAL = 14                     # relu groups on ScalarE (rest on VectorE)
ALPHA = 0.01
EPS = 1e-5

bf16 = ml_dtypes.bfloat16
f8 = ml_dtypes.float8_e4m3

# ---- feature -> (region, partition) map ----
# pair p: region p//32, quadrant p%4, slot (p//4)%8
IDX = np.full((2, 128), -1, np.int64)
for p in range(NPAIR):
    F, q, s = p // 32, p % 4, (p // 4) % 8
    for j in range(2):
        for e in range(2):
            IDX[F, 32 * q + 4 * s + 2 * j + e] = 4 + 4 * p + 2 * j + e
for f in range(4):
    IDX[1, 124 + f] = f

# batch-column permutation: xt column c = bt*512 + 128*s + p holds local
# batch row bt*512 + 4*p + s  (so the logit transpose -> OUT DMA is
# 512B-contiguous per partition)
ROWIDX = np.empty(BL, np.int64)
for bt in range(NBT):
    for s in range(4):
        for pp in range(128):
            ROWIDX[bt * 512 + 128 * s + pp] = bt * 512 + 4 * pp + s

_prog_cache = {}


def _build_host_constants(W_me, b_me, W1, b1, W2, b2, W3, b3):
    # Wall [128, 8*128]: quadrant row 4m+2j+e, block m, cols 64j..64j+63
    # hold W_me[:, e]; replicated to all 4 quadrants.
    pat = np.zeros((32, 8 * 128), np.float32)
    for m in range(8):
        for j in range(2):
            for e in range(2):
                pat[4 * m + 2 * j + e, m * 128 + 64 * j: m * 128 + 64 * (j + 1)] = W_me[:, e]
    Wall = np.tile(pat, (4, 1))

    sel = np.zeros((128, 64), np.float32)
    for j in range(2):
        sel[np.arange(64) + 64 * j, np.arange(64)] = 1.0
    selpack = np.concatenate([sel, sel], axis=1).astype(f8)

    # msel_F [128, 2]: partition of (pair, j, e) contributes to m_raw[e]
    msel = np.zeros((2, 128, 2), np.float32)
    for p in range(NPAIR):
        F, q, s = p // 32, p % 4, (p // 4) % 8
        for j in range(2):
            for e in range(2):
                msel[F, 32 * q + 4 * s + 2 * j + e, e] = 1.0

    W1h = W1[:, :4]                       # [64, 4]
    W1b = W1[:, 4:]                       # [64, 64]
    M2 = (ALPHA / K_ENT) * (W1b @ W_me)   # [64, 2]

    # layer-1 lhsTs (ps = lhsT.T @ rhs)
    L_R1 = (((1.0 - ALPHA) / K_ENT) * W1b).T          # [64, 64]
    L_A1 = msel[0] @ M2.T                             # [128, 64]
    L_B1 = msel[1] @ M2.T                             # [128, 64]
    for f in range(4):
        L_B1[124 + f, :] += W1h[:, f]
    b1eff = b1 + ALPHA * (W1b @ b_me)

    aW2T = (ALPHA * W2).T
    L_R2 = L_R1 @ aW2T
    L_A2 = L_A1 @ aW2T
    L_B2 = L_B1 @ aW2T
    L_r1 = ((1.0 - ALPHA) * W2).T                     # [64, 64]
    b2eff = ALPHA * (W2 @ b1eff) + b2

    aW3T = (ALPHA * W3).T
    L_R3 = L_R2 @ aW3T                                # [64, 32]
    L_A3 = L_A2 @ aW3T                                # [128, 32]
    L_B3 = L_B2 @ aW3T
    L_r1_3 = L_r1 @ aW3T                              # [64, 32]
    L_r2 = ((1.0 - ALPHA) * W3).T                     # [64, 32]
    b3eff = ALPHA * (W3 @ b2eff) + b3

    # packed consts
    # CB16 [128, 1024 + 64+64+32 + 64+64+32 = 1344]
    CB16 = np.zeros((128, 1344), np.float32)
    CB16[:, 0:1024] = Wall
    CB16[:, 1024:1088] = L_A1
    CB16[:, 1088:1152] = L_A2
    CB16[:, 1152:1184] = L_A3
    CB16[:, 1184:1248] = L_B1
    CB16[:, 1248:1312] = L_B2
    CB16[:, 1312:1344] = L_B3
    CB16 = CB16.astype(bf16)

    # CF32 [64, 64+64+32+64+32+32 = 288]  (f32r lhsTs)
    CF32 = np.zeros((64, 288), np.float32)
    CF32[:, 0:64] = L_R1
    CF32[:, 64:128] = L_R2
    CF32[:, 128:160] = L_R3
    CF32[:, 160:224] = L_r1
    CF32[:, 224:256] = L_r1_3
    CF32[:, 256:288] = L_r2

    # CSCAL [128, 4]: bvec | b1eff | b2eff | b3eff
    CSCAL = np.zeros((128, 4), np.float32)
    CSCAL[:, 0] = np.tile(b_me, 2)
    CSCAL[0:64, 1] = b1eff
    CSCAL[0:64, 2] = b2eff
    CSCAL[0:32, 3] = b3eff

    ident32 = np.eye(32, dtype=np.float32)

    return dict(CB16=CB16, CF32=CF32, CSCAL=CSCAL, IDENT32=ident32,
                SELP=selpack)


# const slices (columns in packed tensors)
CB_WALL = 0
CB_A = {1: 1024, 2: 1088, 3: 1152}
CB_B = {1: 1184, 2: 1248, 3: 1312}
CF_R = {1: 0, 2: 64, 3: 128}
CF_r1 = 160
CF_r1_3 = 224
CF_r2 = 256


def build_program(num_devices=NCORES):
    from contextlib import ExitStack
    import concourse.bass as bass
    import concourse.bacc as bacc
    import concourse.tile as tile
    from concourse import mybir

    fp32 = mybir.dt.float32
    bf16d = mybir.dt.bfloat16
    fp8d = mybir.dt.float8e4
    f32r = mybir.dt.float32r
    ALU = mybir.AluOpType
    ACTF = mybir.ActivationFunctionType

    nc = bacc.Bacc(None, num_devices=num_devices)

    XT = nc.declare_dram_parameter("XT", [128, 2, BL], bf16d, isOutput=False)
    OUT = nc.declare_dram_parameter("OUT", [BL, A], fp32, isOutput=True)
    CB16 = nc.declare_dram_parameter("CB16", [128, 1344], bf16d, isOutput=False)
    CF32 = nc.declare_dram_parameter("CF32", [64, 288], f32r, isOutput=False)
    CSCAL = nc.declare_dram_parameter("CSCAL", [128, 4], fp32, isOutput=False)
    IDENT32 = nc.declare_dram_parameter("IDENT32", [32, 32], fp32, isOutput=False)
    SELP = nc.declare_dram_parameter("SELP", [128, 128], fp8d, isOutput=False)

    with tile.TileContext(nc) as tc, ExitStack() as ctx:
        singles = ctx.enter_context(tc.tile_pool(name="singles", bufs=1))
        xtp = ctx.enter_context(tc.tile_pool(name="xtp", bufs=1))
        statp = ctx.enter_context(tc.tile_pool(name="statp", bufs=1))
        dramp = ctx.enter_context(tc.tile_pool(name="dramp", bufs=1, space="DRAM"))

        # ---- loads ----
        xt = xtp.tile([128, 2, BL], bf16d)
        for F in range(2):
            nc.sync.dma_start(out=xt[:, F, :], in_=XT[:, F, :])

        cb = singles.tile([128, 1344], bf16d, tag="cb")
        nc.sync.dma_start(out=cb[:], in_=CB16[:])
        cf = singles.tile([64, 288], f32r, tag="cf")
        nc.sync.dma_start(out=cf[:], in_=CF32[:])
        csc = singles.tile([128, 4], fp32, tag="csc")
        nc.sync.dma_start(out=csc[:], in_=CSCAL[:])
        id32 = singles.tile([32, 32], fp32, tag="id32")
        nc.sync.dma_start(out=id32[:], in_=IDENT32[:])
        selp = singles.tile([128, 128], fp8d, tag="selp")
        nc.sync.dma_start(out=selp[:], in_=SELP[:])

        bvec = csc[:, 0:1]
        b1eff = csc[0:64, 1:2]
        b2eff = csc[0:64, 2:3]
        b3eff = csc[0:32, 3:4]

        # ---- local stats:  stt = [sxA, sqA, sxB, sqB]  [128, 4] ----
        junk = statp.tile([128, BL], bf16d, tag="junk")
        stt = statp.tile([128, 4], fp32, tag="stt")
        junk2 = statp.tile([128, BL], bf16d, tag="junk2")
        for F in range(2):
            nc.vector.tensor_scalar(out=junk[:], in0=xt[:, F, :], scalar1=1.0,
                                    scalar2=None, op0=ALU.mult, op1=ALU.add,
                                    accum_out=stt[:, 2 * F:2 * F + 1])
            nc.scalar.activation(junk2[:], xt[:, F, :], ACTF.Square,
                                 accum_out=stt[:, 2 * F + 1:2 * F + 2])

        cc_in = dramp.tile([128, 4], fp32, tag="cc_in")
        cc_out = dramp.tile([128, 4], fp32, tag="cc_out")
        nc.gpsimd.dma_start(out=cc_in[:], in_=stt[:])
        nc.gpsimd.collective_compute(
            "AllReduce", ALU.add,
            replica_groups=[list(range(num_devices))],
            ins=[cc_in[:].opt()], outs=[cc_out[:].opt()],
        )
        st2 = statp.tile([128, 4], fp32, tag="st2")
        nc.gpsimd.dma_start(out=st2[:], in_=cc_out[:])

        # preload ACT sqrt table during the AllReduce window
        dummy = statp.tile([1, 1], fp32, tag="dummy")
        nc.scalar.activation(dummy[:], id32[0:1, 0:1], ACTF.Sqrt)

        # HAM warmup matmuls during the AllReduce window (junk results).
        # Allocated from mlpp so the bank is recycled by the MLP tail.
        mlpp = ctx.enter_context(tc.tile_pool(name="mlpp", bufs=1, space="PSUM"))
        warm = mlpp.tile([64, 512], fp32, tag="mlp")
        NWARM = 24
        for w in range(NWARM):
            nc.tensor.matmul(warm[:], cb[:, 0:64], cb[:, 0:512],
                             start=(w == 0), stop=(w == NWARM - 1))

        # ---- stats chain ----
        muex = statp.tile([128, 4], fp32, tag="muex")
        nc.vector.tensor_scalar(out=muex[:], in0=st2[:], scalar1=1.0 / B_FULL,
                                scalar2=None, op0=ALU.mult)
        mus = muex[:, 0:4:2]
        ex2 = muex[:, 1:4:2]
        mu2 = statp.tile([128, 2], fp32, tag="mu2")
        nc.vector.tensor_tensor(out=mu2[:], in0=mus, in1=mus, op=ALU.mult)
        vpe = statp.tile([128, 2], fp32, tag="vpe")
        nc.vector.scalar_tensor_tensor(out=vpe[:], in0=mu2[:], scalar=-1.0,
                                       in1=ex2, op0=ALU.mult, op1=ALU.add)
        nc.vector.tensor_scalar(out=vpe[:], in0=vpe[:], scalar1=EPS, scalar2=None,
                                op0=ALU.add)
        rs = statp.tile([128, 2], fp32, tag="rs")
        nc.vector.reciprocal(rs[:], vpe[:])
        svec = statp.tile([128, 2], fp32, tag="sv")
        nc.scalar.activation(svec[:], rs[:], ACTF.Sqrt)
        nmvec = statp.tile([128, 2], fp32, tag="nm")
        nc.vector.scalar_tensor_tensor(out=nmvec[:], in0=mus, scalar=-1.0,
                                       in1=svec[:], op0=ALU.mult, op1=ALU.mult)
        # preload ACT exp table right after the last sqrt use
        nc.scalar.activation(dummy[:], id32[0:1, 0:1], ACTF.Exp)

        # normalize xt in place
        for F in range(2):
            nc.vector.tensor_scalar(
                out=xt[:, F, :], in0=xt[:, F, :],
                scalar1=svec[:, F:F + 1], scalar2=nmvec[:, F:F + 1],
                op0=ALU.mult, op1=ALU.add)

        # ---- main loop ----
        zpool = ctx.enter_context(tc.tile_pool(name="zpool", bufs=3, space="PSUM"))
        accp = ctx.enter_context(tc.tile_pool(name="accp", bufs=1, space="PSUM"))
        ypool = ctx.enter_context(tc.tile_pool(name="ypool", bufs=3))
        mlps = ctx.enter_context(tc.tile_pool(name="mlps", bufs=6))
        outp = ctx.enter_context(tc.tile_pool(name="outp", bufs=4))

        selp3 = selp[:].rearrange("p (two f) -> p two f", two=2)

        # engine pattern: NSCAL scalar groups spread among NGRP
        eng = []
        acc_s = 0
        for gi in range(NGRP):
            ns = ((gi + 1) * NSCAL) // NGRP
            eng.append('S' if ns > acc_s else 'V')
            acc_s = ns

        for bt in range(NBT):
            col0 = bt * NT
            acc = accp.tile([64, NT], fp32, tag="acc")
            ytiles = {}

            def emit_z(gi, _col0=col0):
                zps = zpool.tile([128, GW * NT], fp32, tag="z")
                for j in range(GW):
                    p = gi * GW + j
                    q, m, F = p % 4, (p // 4) % 8, p // 32
                    nc.tensor.matmul(
                        zps[:, j * NT:(j + 1) * NT],
                        cb[32 * q:32 * (q + 1), 128 * m:128 * (m + 1)],
                        xt[32 * q:32 * (q + 1), F, _col0:_col0 + NT],
                        start=True, stop=True,
                        tile_position=(32 * int(q), 0),
                    )
                return zps

            ztiles = {}

            def emit_relu(gi):
                zps = ztiles.pop(gi)
                y = ypool.tile([128, GW * NT], fp8d)
                ytiles[gi] = y
                if eng[gi] == 'S':
                    nc.scalar.activation(y[:], zps[:], ACTF.Relu, bias=bvec,
                                         scale=1.0)
                else:
                    nc.vector.tensor_scalar(out=y[:], in0=zps[:], scalar1=bvec,
                                            scalar2=0.0, op0=ALU.add, op1=ALU.max)

            def emit_pool(gi, _acc=acc):
                y = ytiles.pop(gi)
                y3 = y.rearrange("p (j d) -> p j d", j=GW)
                nc.tensor.matmul(_acc[:], selp3, y3[:, 0:GW, :],
                                 start=(gi == 0), stop=(gi == NGRP - 1),
                                 perf_mode=mybir.MatmulPerfMode.DoubleRow)

            for step in range(NGRP + 2):
                if step < NGRP:
                    ztiles[step] = emit_z(step)
                if step >= 1 and step - 1 < NGRP:
                    emit_relu(step - 1)
                if step >= 2:
                    emit_pool(step - 2)

            # ---- MLP tail (alpha paths folded into direct matmuls) ----
            polR = mlps.tile([64, NT], f32r, tag="polR")
            nc.vector.tensor_copy(polR[:], acc[:])

            xtA = xt[:, 0, col0:col0 + NT]
            xtB = xt[:, 1, col0:col0 + NT]

            ps_h1 = mlpp.tile([64, NT], fp32, tag="mlp")
            nc.tensor.matmul(ps_h1[:], cf[:, CF_R[1]:CF_R[1] + 64], polR[:],
                             start=True, stop=False)
            nc.tensor.matmul(ps_h1[:], cb[:, CB_A[1]:CB_A[1] + 64], xtA,
                             start=False, stop=False)
            nc.tensor.matmul(ps_h1[:], cb[:, CB_B[1]:CB_B[1] + 64], xtB,
                             start=False, stop=True)
            r1 = mlps.tile([64, NT], f32r, tag="r1")
            nc.scalar.activation(r1[:], ps_h1[:], ACTF.Relu, bias=b1eff)

            ps_h2 = mlpp.tile([64, NT], fp32, tag="mlp")
            nc.tensor.matmul(ps_h2[:], cf[:, CF_R[2]:CF_R[2] + 64], polR[:],
                             start=True, stop=False)
            nc.tensor.matmul(ps_h2[:], cb[:, CB_A[2]:CB_A[2] + 64], xtA,
                             start=False, stop=False)
            nc.tensor.matmul(ps_h2[:], cb[:, CB_B[2]:CB_B[2] + 64], xtB,
                             start=False, stop=False)
            nc.tensor.matmul(ps_h2[:], cf[:, CF_r1:CF_r1 + 64], r1[:],
                             start=False, stop=True)
            r2 = mlps.tile([64, NT], f32r, tag="r2")
            nc.scalar.activation(r2[:], ps_h2[:], ACTF.Relu, bias=b2eff)

            ps_lg = mlpp.tile([32, NT], fp32, tag="mlp")
            nc.tensor.matmul(ps_lg[:], cf[:, CF_R[3]:CF_R[3] + 32], polR[:],
                             start=True, stop=False)
            nc.tensor.matmul(ps_lg[:], cb[:, CB_A[3]:CB_A[3] + 32], xtA,
                             start=False, stop=False)
            nc.tensor.matmul(ps_lg[:], cb[:, CB_B[3]:CB_B[3] + 32], xtB,
                             start=False, stop=False)
            nc.tensor.matmul(ps_lg[:], cf[:, CF_r1_3:CF_r1_3 + 32], r1[:],
                             start=False, stop=False)
            nc.tensor.matmul(ps_lg[:], cf[:, CF_r2:CF_r2 + 32], r2[:],
                             start=False, stop=True)
            lg = mlps.tile([32, NT], fp32, tag="lg")
            nc.vector.tensor_scalar(out=lg[:], in0=ps_lg[:], scalar1=b3eff,
                                    scalar2=None, op0=ALU.add)

            # ---- softmax (transpose to batch-major) ----
            ps_tr = mlpp.tile([128, 128], fp32, tag="mlp")
            for s in range(4):
                nc.tensor.transpose(ps_tr[:, 32 * s:32 * (s + 1)],
                                    lg[:, 128 * s:128 * (s + 1)], id32[:])
            esb = outp.tile([128, 128], fp32, tag="e")
            nc.scalar.activation(esb[:], ps_tr[:], ACTF.Exp)
            e3 = esb.rearrange("p (s a) -> p s a", s=4)
            sums = outp.tile([128, 4], fp32, tag="sums")
            nc.vector.tensor_reduce(out=sums[:], in_=e3[:, :, :],
                                    axis=mybir.AxisListType.X, op=ALU.add)
            rec = outp.tile([128, 4], fp32, tag="rec")
            nc.vector.reciprocal(rec[:], sums[:])
            fin = outp.tile([128, 128], fp32, tag="fin")
            fin3 = fin.rearrange("p (s a) -> p s a", s=4)
            rec_b = rec[:].unsqueeze(2).broadcast_to([128, 4, 32])
            nc.vector.tensor_tensor(out=fin3[:, :, :], in0=e3[:, :, :], in1=rec_b,
                                    op=ALU.mult)
            # row r = bt*512 + 4*p + s  (per the host ROWIDX permutation)
            oap = OUT[:]
            oout = bass.AP(
                tensor=oap.tensor, offset=oap.offset + bt * NT * A,
                ap=[[4 * A, 128], [A, 4], [1, A]],
            )
            nc.sync.dma_start(out=oout, in_=fin3[:, :, :])
    nc.finalize()
    return nc


def make_in_maps(inputs):
    X = np.asarray(inputs["X"], np.float32)
    consts = _build_host_constants(
        np.asarray(inputs["W_me"], np.float32), np.asarray(inputs["b_me"], np.float32),
        np.asarray(inputs["W1"], np.float32), np.asarray(inputs["b1"], np.float32),
        np.asarray(inputs["W2"], np.float32), np.asarray(inputs["b2"], np.float32),
        np.asarray(inputs["W3"], np.float32), np.asarray(inputs["b3"], np.float32),
    )
    Xb = X.astype(bf16)
    # XT [core, 128, 2, BL]
    XT = np.zeros((NCORES, 128, 2, BL), bf16)
    for F in range(2):
        for prt in range(128):
            f = IDX[F, prt]
            if f >= 0:
                XT[:, prt, F, :] = Xb[:, f].reshape(NCORES, BL)[:, ROWIDX]
    in_maps = []
    for i in range(NCORES):
        m = {"XT": np.ascontiguousarray(XT[i])}
        m.update(consts)
        in_maps.append(m)
    return in_maps


def kernel(**inputs):
    from concourse.bass_utils import run_bass_kernel_spmd

    if "nc" not in _prog_cache:
        _prog_cache["nc"] = build_program(NCORES)
    nc = _prog_cache["nc"]

    in_maps = make_in_maps(inputs)
    res = run_bass_kernel_spmd(nc, in_maps, list(range(NCORES)))
    out = np.concatenate([res.results[i]["OUT"] for i in range(NCORES)], axis=0)
    return out.astype(np.float32)
